# revision 15
# baseline (speedup 1.0000x reference)
"""Trainium2 Bass kernel for nn_Attention_42417097015520.

Full-input contract: kernel(**inputs) takes the unsharded inputs
(x [4,2048,768], W_qkv [768,2304], W_proj [768,768], b_proj [768]) and
returns the full [4,2048,768] output.

Sharding (8 cores): core c handles batch b=c//2 and heads
h in [(c%2)*6, (c%2)*6+6) (tensor parallel over heads x data parallel
over batch). Each core computes its 6 heads' attention plus the partial
output projection against its 384-row slice of W_proj; the host sums the
two partials per batch and adds b_proj.

Device-side layout/algorithm (per core, identical SPMD program, bf16
matmul operands, fp32 PSUM accumulation):
  - inputs: xT = x[b].T [768,2048], wqkv = W_qkv column slice [768,1152]
    (q|k|v blocks of 384), wproj row slice [384,768].
  - Q^T/K^T projections in head-pair-packed planes (head h on partitions
    (h%2)*64.. of plane h//2); V plain [m, 6*64].
  - Attention blocks (n-block x head-plane): S^T chunks for both heads of
    a plane issued as row-tiled matmuls at partition bases 0/64 (disjoint
    row groups run concurrently), exp on ScalarE (scale=1/8 folded in,
    two chunks per activation).
  - P@V as a 2x2 tile_position quad per m-chunk: (head x m-half) tiles of
    K=64/M=64 run CONCURRENTLY in the four array quadrants (full PE rate,
    2x the old M=65-padded scheme). X psum gets the m-lo halves (A on
    partitions 0:64, B on 64:128), Y the m-hi halves (B low, A high);
    a DVE add folds X+Y into the packed O^T layout.
  - Softmax denominators can no longer ride the P@V matmul (no M slack),
    so they are computed by a bf16 DVE pairwise-add tree over the P^T
    slots (2x_1P mode, round 1 interleaved with the attention quads) plus
    a single ones-vector matmul for the final 128-partition reduction;
    reciprocal is spread/broadcast via the DRAM bounce as before and
    applied to X+Y in one [128,512] multiply.
  - All remaining production (K/V/Q projections, output projection) is
    drained as "extra PE work" between attention groups from a JIT queue,
    and each block's first S-matmuls are emitted ahead of the previous
    block's tail, so ScalarE (the ~200us exp floor) and the PE stay busy
    simultaneously.
  - Output projection from the O^T layout (heads on partitions), partial
    result [2048,768] DMA'd out; host sums batch partials + b_proj.
"""

import sys
import types
import contextlib
import ctypes
from contextlib import ExitStack

import numpy as np

import concourse.bass as bass
import concourse.mybir as mybir
import concourse.tile as tile
from concourse.bass_utils import run_bass_kernel_spmd

B, N, D, H, HD = 4, 2048, 768, 12, 64
HPC = H // 2          # heads per core = 6
NCORES = 8
SCALE = HD ** -0.5    # 0.125
F32 = mybir.dt.float32
F32R = mybir.dt.float32r
BF16 = mybir.dt.bfloat16
P = 128
VW = HD + 1           # V columns per head incl. ones column = 65


# ---------------------------------------------------------------------------
# Workaround: this container's walrus accepts at most ONE sem wait per
# instruction. Hoist extra waits onto same-engine NoOps inserted before.
# ---------------------------------------------------------------------------
_wsplit_ctr = [0]


def _split_waits(nc, cap: int = 1) -> int:
    n_split = 0
    for f in nc.m.functions:
        for bb in f.blocks:
            insts = list(bb.instructions)
            out = []
            for ins in insts:
                si = ins.sync_info
                if si is not None and si.on_wait and len(si.on_wait) > cap:
                    waits = list(si.on_wait)
                    for i in range(0, len(waits) - cap, cap):
                        _wsplit_ctr[0] += 1
                        out.append(
                            mybir.InstNoOp(
                                name=f"I-wsplit-{_wsplit_ctr[0]}",
                                engine=ins.engine,
                                ins=[],
                                outs=[],
                                sync_info=mybir.SyncInfo(
                                    on_wait=waits[i : i + cap], on_update=[]
                                ),
                            )
                        )
                    si.on_wait = waits[len(waits) - cap :]
                    n_split += 1
                out.append(ins)
            if len(out) != len(insts):
                bb.instructions[:] = out
    return n_split


# ---------------------------------------------------------------------------
# NTFF profiling shim (the image's antenv lacks axon_hooks); only needed
# when trace=True is requested.
# ---------------------------------------------------------------------------
_HOOK = [None]


def _install_ntff_shim():
    if "antenv.axon_hooks" in sys.modules:
        return
    mod = types.ModuleType("antenv.axon_hooks")
    mod.set_axon_ntff_profile_hook = lambda h: _HOOK.__setitem__(0, h)
    mod.get_axon_ntff_profile_hook = lambda: _HOOK[0]
    sys.modules["antenv.axon_hooks"] = mod
    try:
        import antenv

        antenv.axon_hooks = mod
    except ImportError:
        pass

    try:
        lib = ctypes.CDLL("/opt/axon/libaxon_pjrt.so")
    except OSError:
        return
    if not hasattr(lib, "axon_start_nrt_profile"):
        return
    lib.axon_start_nrt_profile.argtypes = [
        ctypes.POINTER(ctypes.c_int64),
        ctypes.c_size_t,
    ]
    lib.axon_start_nrt_profile.restype = ctypes.c_int64
    lib.axon_stop_nrt_profile.argtypes = [ctypes.c_char_p]
    lib.axon_stop_nrt_profile.restype = ctypes.c_int64

    @contextlib.contextmanager
    def _hook(output_dir, device_ids):
        import jax

        jax.devices()
        if device_ids:
            ids = (ctypes.c_int64 * len(device_ids))(*device_ids)
            rc = lib.axon_start_nrt_profile(ids, len(device_ids))
        else:
            rc = lib.axon_start_nrt_profile(None, 0)
        if rc != 0:
            raise RuntimeError(f"axon_start_nrt_profile rc={rc}")
        try:
            yield
        finally:
            n = lib.axon_stop_nrt_profile(str(output_dir).encode())
            if n < 0:
                raise RuntimeError(f"axon_stop_nrt_profile rc={n}")

    _HOOK[0] = _hook

    import concourse.bass_utils as bu

    bu.upload_artifacts = lambda tmpdir: str(tmpdir)


# ---------------------------------------------------------------------------
# Device program
# ---------------------------------------------------------------------------
def _build_nc():
    nc = bass.Bass()
    xT = nc.declare_dram_parameter("xT", [D, N], BF16, isOutput=False).ap()
    wqkv = nc.declare_dram_parameter("wqkv", [D, 3 * HPC * HD], BF16, isOutput=False).ap()
    wproj = nc.declare_dram_parameter("wproj", [HPC * HD, D], BF16, isOutput=False).ap()
    out = nc.declare_dram_parameter("out", [N, D], F32, isOutput=True).ap()

    DO = D // P          # 6 d-chunks of 128
    NB = N // 512        # 4 n-blocks of 512
    MC = N // P          # 16 m-chunks of 128
    PH = HPC * HD // P   # 3 planes of head-dims

    with tile.TileContext(nc) as tc, ExitStack() as ctx:
        persist = ctx.enter_context(tc.tile_pool(name="persist", bufs=1))
        ptp = ctx.enter_context(tc.tile_pool(name="ptp", bufs=2))
        redp = ctx.enter_context(tc.tile_pool(name="redp", bufs=1))
        outcp = ctx.enter_context(tc.tile_pool(name="outcp", bufs=3))
        small = ctx.enter_context(tc.tile_pool(name="small", bufs=2))
        dramp = ctx.enter_context(tc.tile_pool(name="dramp", bufs=4, space="DRAM"))
        psum_mm = ctx.enter_context(tc.tile_pool(name="psum_mm", bufs=2, space="PSUM"))
        psum_s = ctx.enter_context(tc.tile_pool(name="psum_s", bufs=2, space="PSUM"))
        psum_o = ctx.enter_context(tc.tile_pool(name="psum_o", bufs=2, space="PSUM"))

        # Q^T and K^T planes use head-pair packing: head h lives on
        # partitions (h%2)*64.. of plane h//2. The S^T matmuls for the two
        # heads of a plane are emitted back-to-back as row-tiled (base
        # partition 0 / 64) matmuls, so they run CONCURRENTLY on disjoint
        # row groups of the PE array -- 2x throughput, and the combined
        # activity keeps the HAM clock-gate at full speed.
        qT_sb = persist.tile([P, PH, N], BF16)                   # [128, 3, 2048]
        kT_sb = persist.tile([P, PH, N], BF16)                   # [128, 3, 2048]
        v_sb = persist.tile([P, MC, HPC * HD], BF16)             # [128, 16, 384]
        oT_sb = persist.tile([P, PH, N], BF16)                   # [128, 3, 2048]
        wp_sb = persist.tile([P, PH, D], BF16)                   # [128, 3, 768]
        xT_sb = persist.tile([P, DO, N], BF16)                   # [128, 6, 2048]
        wqkv_sb = persist.tile([P, DO, 3 * HPC * HD], BF16)      # [128, 6, 1152]
        ones_sb = persist.tile([P, 1], BF16)                     # ones lhsT column

        nc.vector.memset(ones_sb[:, :], 1.0)

        QK = 2 * HPC * HD
        for o in range(DO):
            nc.gpsimd.dma_start(out=wqkv_sb[:, o, 0:QK], in_=wqkv[o * P:(o + 1) * P, 0:QK])
            nc.sync.dma_start(out=xT_sb[:, o, 0:512], in_=xT[o * P:(o + 1) * P, 0:512])
        for o in range(DO):
            nc.gpsimd.dma_start(out=wqkv_sb[:, o, QK:], in_=wqkv[o * P:(o + 1) * P, QK:])
        for o in range(DO):
            nc.sync.dma_start(out=xT_sb[:, o, 512:N // 2], in_=xT[o * P:(o + 1) * P, 512:N // 2])
        for o in range(DO):
            nc.sync.dma_start(out=xT_sb[:, o, N // 2:N], in_=xT[o * P:(o + 1) * P, N // 2:N])
        for p3 in range(PH):
            nc.sync.dma_start(out=wp_sb[:, p3, :], in_=wproj[p3 * P:(p3 + 1) * P, :])

        # qkv production units are split in halves (3 accumulating matmuls
        # each, ~1.2us) so the JIT drain stays fine-grained and never delays
        # the next S-pair by a full 2.4us
        def qk_proj_half(cb, nb, half, box):
            if half == 0:
                box["ps"] = psum_mm.tile([P, 512], F32, tag="mmps", name="qkps")
            ps = box["ps"]
            for o in range(3 * half, 3 * half + 3):
                nc.tensor.matmul(
                    ps[:, :],
                    lhsT=wqkv_sb[:, o, cb * P:(cb + 1) * P],
                    rhs=xT_sb[:, o, nb * 512:(nb + 1) * 512],
                    start=(o == 0),
                    stop=(o == DO - 1),
                )
            if half == 1:
                sl = slice(nb * 512, (nb + 1) * 512)
                if cb < PH:
                    nc.vector.tensor_copy(qT_sb[:, cb, sl], ps[:, :])
                else:
                    nc.vector.tensor_copy(kT_sb[:, cb - PH, sl], ps[:, :])

        def qk_proj(cb, nb):
            box = {}
            qk_proj_half(cb, nb, 0, box)
            qk_proj_half(cb, nb, 1, box)

        def v_proj_half(mc, half, box):
            if half == 0:
                box["ps"] = psum_mm.tile([P, 512], F32, tag="mmps", name="vps")
            ps = box["ps"]
            for o in range(3 * half, 3 * half + 3):
                nc.tensor.matmul(
                    ps[:, : HPC * HD],
                    lhsT=xT_sb[:, o, mc * P:(mc + 1) * P],
                    rhs=wqkv_sb[:, o, 2 * HPC * HD: 3 * HPC * HD],
                    start=(o == 0),
                    stop=(o == DO - 1),
                )
            if half == 1:
                nc.vector.tensor_copy(v_sb[:, mc, :], ps[:, : HPC * HD])

        def v_proj(mc):
            box = {}
            v_proj_half(mc, 0, box)
            v_proj_half(mc, 1, box)

        def proj(nb):
            """Output projection for one 512-row n-block."""
            for mcl in range(512 // P):
                mc = nb * (512 // P) + mcl
                for half in range(2):
                    ps = psum_mm.tile([P, 512], F32, tag="mmps")
                    for p3 in range(PH):
                        nc.tensor.matmul(
                            ps[:, : D // 2],
                            lhsT=oT_sb[:, p3, mc * P:(mc + 1) * P],
                            rhs=wp_sb[:, p3, half * (D // 2):(half + 1) * (D // 2)],
                            start=(p3 == 0),
                            stop=(p3 == PH - 1),
                        )
                    oc = outcp.tile([P, D // 2], F32)
                    nc.vector.tensor_copy(oc[:, :], ps[:, : D // 2])
                    nc.sync.dma_start(
                        out=out[mc * P:(mc + 1) * P,
                                half * (D // 2):(half + 1) * (D // 2)],
                        in_=oc[:, :],
                    )

        def s_pair(nb, hp, mc):
            """S^T chunk mc for BOTH heads of plane hp: two K=64 row-tiled
            matmuls at partition bases 0 and 64 -- concurrent on the PE."""
            ps = psum_s.tile([P, 1024], F32, tag="sps")
            for j in range(2):
                kb = j * HD
                nc.tensor.matmul(
                    ps[:, j * 512:(j + 1) * 512],
                    lhsT=kT_sb[kb:kb + HD, hp, mc * P:(mc + 1) * P],
                    rhs=qT_sb[kb:kb + HD, hp, nb * 512:(nb + 1) * 512],
                    start=True,
                    stop=True,
                    tile_position=(kb, 0),
                )
            return ps

        # ---- minimal serial prologue, everything else drained JIT ----
        # kTz heads 0/1 for m<512 and Q^T plane 0 for n-block 0 are all the
        # first attention groups need; the rest of the K/V/Q production is
        # queued and emitted between attention groups so the PE produces
        # while ScalarE works through the exps.
        qk_proj(PH, 0)
        qk_proj(0, 0)

        extraq = []

        def drain(k):
            for _ in range(k):
                if extraq:
                    extraq.pop(0)()

        def push_qk(cb, nb):
            box = {}
            extraq.append(lambda: qk_proj_half(cb, nb, 0, box))
            extraq.append(lambda: qk_proj_half(cb, nb, 1, box))

        def push_v(mc):
            box = {}
            extraq.append(lambda: v_proj_half(mc, 0, box))
            extraq.append(lambda: v_proj_half(mc, 1, box))

        for mc in range(3):
            push_v(mc)
        push_qk(PH, 1)
        for mc in range(3, 5):
            push_v(mc)
        push_qk(0, 1)
        for mc in range(5, 7):
            push_v(mc)
        push_qk(PH, 2)
        for mc in range(7, 10):
            push_v(mc)
        push_qk(PH, 3)
        for mc in range(10, MC):
            push_v(mc)
        push_qk(0, 2)
        push_qk(0, 3)

        def proj_unit(nb, mcl, half):
            mc = nb * (512 // P) + mcl
            ps = psum_mm.tile([P, 512], F32, tag="mmps")
            p3s = (0, 1, 2)
            for i, p3 in enumerate(p3s):
                nc.tensor.matmul(
                    ps[:, : D // 2],
                    lhsT=oT_sb[:, p3, mc * P:(mc + 1) * P],
                    rhs=wp_sb[:, p3, half * (D // 2):(half + 1) * (D // 2)],
                    start=(i == 0),
                    stop=(i == PH - 1),
                )
            oc = outcp.tile([P, D // 2], F32)
            nc.vector.tensor_copy(oc[:, :], ps[:, : D // 2])
            nc.sync.dma_start(
                out=out[mc * P:(mc + 1) * P,
                        half * (D // 2):(half + 1) * (D // 2)],
                in_=oc[:, :],
            )

        # ---- attention: software-pipelined (nb, head-plane) blocks;
        # each block handles BOTH heads of one Q/K plane ----
        blocks = [(nb, hp) for hp in range(PH) for nb in range(NB)]
        pending = None
        deferred = []

        def finish_block(nb, hp, oU, dpart):
            """Denominator matmuls + reciprocal broadcast + normalization for
            a completed block. Deferred into the NEXT block's loop so the
            dps matmuls (which wait on the DVE tree) never head-of-line
            block the PE queue at a block boundary."""
            rcp2 = small.tile([P, 512], F32, tag="rcp2")
            # ONE psum tile for both heads (head h at free cols 128h..) so the
            # finish makes a single "mmps" ring allocation -- this keeps the
            # half-unit drain ring-adjacency invariant intact
            dps = psum_mm.tile([97, 256], F32, tag="mmps", name="dps")
            for h in range(2):
                for q in range(4):
                    nc.tensor.matmul(
                        dps[32 * q:32 * q + 1, 128 * h:128 * (h + 1)],
                        lhsT=ones_sb[:, :],
                        rhs=dpart[:, h, 128 * q:128 * (q + 1)],
                        start=True, stop=True, tile_position=(0, 32 * q),
                    )
            rcp4 = small.tile([97, 256], F32, tag="rcp4")
            nc.vector.reciprocal(rcp4[:, :], dps[:, :])
            d4 = dramp.tile([97, 256], F32, tag="d4")
            nc.sync.dma_start(out=d4[:, :], in_=rcp4[:, :])
            for h in range(2):
                nc.sync.dma_start(
                    out=rcp2[h * HD:(h + 1) * HD, :],
                    in_=bass.AP(tensor=d4.tensor, offset=d4.offset + 128 * h,
                                ap=[[0, HD], [32 * 256, 4], [1, 128]]),
                )
            nc.vector.tensor_mul(
                oT_sb[:, hp, nb * 512:(nb + 1) * 512], oU[:, :], rcp2[:, :]
            )
            # output projection for a completed n-block becomes available
            # only now (it reads the normalized plane)
            if hp == PH - 1:
                for mcl in range(512 // P):
                    for half in range(2):
                        extraq.append(
                            lambda nbp=nb, mcl=mcl, half=half: proj_unit(nbp, mcl, half)
                        )
        def r1_bundle(pT, r1, j0):
            """Denominator tree round 1 for slot pairs j0, j0+1 (slots
            2*j0 .. 2*j0+3), both heads, one strided bf16 2x op."""
            pTe = pT.rearrange("p h (e two) n -> p h e two n", two=2)
            nc.vector.tensor_add(
                r1[:, :, j0:j0 + 2, :],
                pTe[:, :, j0:j0 + 2, 0, :],
                pTe[:, :, j0:j0 + 2, 1, :],
            )

        for bi, (nb, hp) in enumerate(blocks):
            pT = ptp.tile([P, 2, MC, 512], BF16)    # [m, head-in-pair, mc, n]
            r1 = redp.tile([P, 2, MC // 2, 512], BF16, tag="r1")
            X = psum_o.tile([P, 512], F32, tag="po")   # m-lo: A on 0:64, B on 64:
            Y = psum_o.tile([P, 512], F32, tag="po")   # m-hi: A on 0:64, B on 64:
            hA, hB = 2 * hp, 2 * hp + 1

            def o_quad(mc, first):
                last = (mc == MC - 1)
                # four concurrent 64x64 tiles: (head x m-half) quadrants;
                # A stays on cols 0:64 for both halves so X+Y is swizzle-free
                nc.tensor.matmul(
                    X[0:HD, :], lhsT=v_sb[0:HD, mc, hA * HD:(hA + 1) * HD],
                    rhs=pT[0:HD, 0, mc, :], start=first, stop=last,
                    tile_position=(0, 0),
                )
                nc.tensor.matmul(
                    X[HD:P, :], lhsT=v_sb[0:HD, mc, hB * HD:(hB + 1) * HD],
                    rhs=pT[0:HD, 1, mc, :], start=first, stop=last,
                    tile_position=(0, HD),
                )
                nc.tensor.matmul(
                    Y[0:HD, :], lhsT=v_sb[HD:P, mc, hA * HD:(hA + 1) * HD],
                    rhs=pT[HD:P, 0, mc, :], start=first, stop=last,
                    tile_position=(HD, 0),
                )
                nc.tensor.matmul(
                    Y[HD:P, :], lhsT=v_sb[HD:P, mc, hB * HD:(hB + 1) * HD],
                    rhs=pT[HD:P, 1, mc, :], start=first, stop=last,
                    tile_position=(HD, HD),
                )

            # quads: two S-pairs back-to-back so the second pair's weight
            # loads hide under the first pair's streams
            for mc2 in range(0, MC, 2):
                if mc2 == 0 and pending is not None:
                    ps0 = pending
                    pending = None
                else:
                    ps0 = s_pair(nb, hp, mc2)
                ps1 = s_pair(nb, hp, mc2 + 1)
                for mc, ps in ((mc2, ps0), (mc2 + 1, ps1)):
                    nc.scalar.activation(
                        pT[:, :, mc, :],
                        ps[:, :],
                        mybir.ActivationFunctionType.Exp,
                        scale=SCALE,
                    )
                ran_deferred = False
                if mc2 == 6 and deferred:
                    for f in deferred:
                        f()
                    deferred.clear()
                    ran_deferred = True
                drain(6 if bi == 0 else (1 if ran_deferred else 2))
                if mc2 >= 2:
                    o_quad(mc2 - 2, first=(mc2 == 2))
                    o_quad(mc2 - 1, first=False)
                if mc2 in (4, 8, 12):
                    r1_bundle(pT, r1, (mc2 - 4) // 2)
                if mc2 == MC - 2 and bi + 1 < len(blocks):
                    # next block's first S-pair ahead of this block's tail,
                    # so ScalarE never starves at the block boundary
                    nb2, hp2 = blocks[bi + 1]
                    pending = s_pair(nb2, hp2, 0)
            o_quad(MC - 2, first=False)
            o_quad(MC - 1, first=False)

            # fold X+Y into the packed O^T layout FIRST (frees the psum
            # banks for the next block's quads before the tree runs on DVE);
            # TT reads at most one PSUM input: copy X, then in-place add Y
            oU = small.tile([P, 512], F32, tag="oU")
            nc.vector.tensor_copy(oU[:, :], X[:, :])
            nc.vector.tensor_add(oU[:, :], oU[:, :], Y[:, :])

            # denominator tree: last round-1 bundle, then rounds 2..4
            # (bf16 2x, both heads per op)
            r1_bundle(pT, r1, MC // 2 - 2)
            r2 = redp.tile([P, 2, MC // 4, 512], BF16, tag="r2")
            r1e = r1.rearrange("p h (e two) n -> p h two e n", two=2)
            nc.vector.tensor_add(r2[:, :, :, :], r1e[:, :, 0, :, :], r1e[:, :, 1, :, :])
            r3 = redp.tile([P, 2, MC // 8, 512], BF16, tag="r3")
            r2e = r2.rearrange("p h (e two) n -> p h two e n", two=2)
            nc.vector.tensor_add(r3[:, :, :, :], r2e[:, :, 0, :, :], r2e[:, :, 1, :, :])
            dpart = redp.tile([P, 2, 512], BF16, tag="r4", bufs=2)
            nc.vector.tensor_add(dpart[:, :, :], r3[:, :, 0, :], r3[:, :, 1, :])

            # the denominator matmuls + normalization are deferred into the
            # next block (they wait on the DVE tree; issuing them now would
            # head-of-line block the PE at the boundary)
            deferred.append(
                lambda nbp=nb, hpp=hp, oUp=oU, dp=dpart: finish_block(nbp, hpp, oUp, dp)
            )

            # queue follow-on PE work: Q/K planes for the next head-plane
            if hp + 1 < PH:
                push_qk(PH + hp + 1, nb)
                push_qk(hp + 1, nb)
        for f in deferred:
            f()
        deferred.clear()
        while extraq:
            extraq.pop(0)()

    _split_waits(nc)
    return nc


_NC_CACHE = [None]


def _get_nc():
    if _NC_CACHE[0] is None:
        _NC_CACHE[0] = _build_nc()
    return _NC_CACHE[0]


def _make_in_maps(x, W_qkv, W_proj):
    import ml_dtypes

    bf16 = ml_dtypes.bfloat16
    in_maps = []
    for c in range(NCORES):
        b = c // 2
        h0 = (c % 2) * HPC
        qcols = W_qkv[:, h0 * HD:(h0 + HPC) * HD]
        kcols = W_qkv[:, D + h0 * HD: D + (h0 + HPC) * HD]
        vcols = W_qkv[:, 2 * D + h0 * HD: 2 * D + (h0 + HPC) * HD]
        in_maps.append(
            {
                "xT": np.ascontiguousarray(x[b].T).astype(bf16),
                "wqkv": np.concatenate([qcols, kcols, vcols], axis=1).astype(bf16),
                "wproj": np.ascontiguousarray(
                    W_proj[h0 * HD:(h0 + HPC) * HD, :]
                ).astype(bf16),
            }
        )
    return in_maps


def _run(inputs, trace=False):
    x = np.asarray(inputs["x"], dtype=np.float32)
    W_qkv = np.asarray(inputs["W_qkv"], dtype=np.float32)
    W_proj = np.asarray(inputs["W_proj"], dtype=np.float32)
    b_proj = np.asarray(inputs["b_proj"], dtype=np.float32)

    if trace:
        _install_ntff_shim()
    nc = _get_nc()
    res = run_bass_kernel_spmd(
        nc, _make_in_maps(x, W_qkv, W_proj), core_ids=list(range(NCORES)),
        trace=trace,
    )
    parts = res.results
    out = np.empty((B, N, D), dtype=np.float32)
    for b in range(B):
        out[b] = parts[2 * b]["out"] + parts[2 * b + 1]["out"] + b_proj
    return out, res


def kernel(**inputs) -> np.ndarray:
    out, _ = _run(inputs, trace=False)
    return out


def run_traced(inputs):
    return _run(inputs, trace=True)

